# revision 5
# baseline (speedup 1.0000x reference)
# Fused conv3x3(same) + bias + tanh + x2 + stride-4 subsample, data-parallel
# over 8 NeuronCores.
#
# Math: out[b,oc,y,x] = 2*tanh(sum_{ic,ky,kx} w[oc,ic,ky,kx]*x[b,ic,4y+ky-1,4x+kx-1] + bias[oc])
# computed in fp16 like the reference. Since the spatial stride (4) exceeds the
# kernel size (3), every output pixel reads a disjoint 3x3x8 input patch, so the
# conv lowers exactly to a [72 -> 64] GEMM over 64*64 pixels per image. The host
# does the im2col rearrangement (pure data movement, fp16 cast is identical to
# the reference's .astype(float16)); each core runs the GEMM + bias + tanh for
# 4 of the 32 images. The trailing *2 and fp32 cast are exact in either order,
# so they are applied on the host after the fp16 tanh.
#
# Device kernel is hand-scheduled raw bacc. Schedule notes (from perfetto
# trace analysis of the previous version):
#  - HBM->SBUF reads sustain only ~15 GB/s per SDMA engine on a single DGE
#    ring (~245 GB/s aggregate) but ~25 GB/s with more rings in flight, so
#    input slices are spread over all three descriptor-generation rings
#    (Sync HWDGE, Scalar HWDGE, GpSimd SWDGE).
#  - The scalar ACT chain (tanh at 1 col/cycle, (N+352)/1.2GHz per call) is
#    the longest serial engine chain (~9.3us); slice 0 is processed as two
#    512-col halves, each gated on only 2 matmuls, so the chain starts ~3us
#    into the run instead of ~12us.
#  - No PE warm-up matmuls: HAM grants full clock (2.4GHz) after ~4us of
#    sustained PE activity and budgets ~7us of it; burning the budget on
#    warm-ups (as before) re-throttles the real matmuls to 50% duty. Real
#    matmuls at the cold 1.2GHz clock are still faster than waiting.
#  - The bias rides in w row K (patch row K is constant 1.0), so there is
#    no separate bias operand anywhere.
#  - The contraction is zero-padded 72 -> 80 rows: an 80-partition DMA
#    spreads over all 16 SDMA engines.
import sys

import numpy as np

try:
    import concourse.bass as bass  # noqa: F401
except ImportError:
    sys.path.insert(0, "/opt/trn_rl_repo")

import concourse.bass as bass  # noqa: F401
import concourse.bacc as bacc
import concourse.mybir as mybir
from concourse.bass_utils import run_bass_kernel_spmd

N_CORES = 8
B_FULL = 32
B_CORE = B_FULL // N_CORES  # 4 images per core
C_IN = 8
KH = KW = 3
K = C_IN * KH * KW  # 72 contraction
KP = 80  # zero-padded contraction (16-SDMA-engine alignment)
OC = 64
OH = OW = 64
NPIX = OH * OW  # 4096
HALF = NPIX // 2  # 2048
NH = 2 * B_CORE  # 8 half-image pipeline slices
F16 = mybir.dt.float16
F32 = mybir.dt.float32

_PROGRAM = None


def build_program():
    from contextlib import ExitStack

    nc = bacc.Bacc("TRN2")
    xp = nc.dram_tensor("xp", [B_CORE, KP, 2, HALF], F16, kind="ExternalInput")
    w = nc.dram_tensor("w", [KP, OC], F16, kind="ExternalInput")
    y = nc.dram_tensor("y", [NH, 2 * OC, HALF // 2], F16, kind="ExternalOutput")

    with ExitStack() as stack:
        w_tile = stack.enter_context(nc.sbuf_tensor([KP, OC], F16))
        # one buffer per slice -> no buffer-reuse waits
        x_bufs = stack.enter_context(nc.sbuf_tensor([KP, NH, HALF], F16))
        a_bufs = stack.enter_context(nc.sbuf_tensor([2 * OC, NH, HALF // 2], F16))
        # 8 banks of [128, 512]; slice i accumulates into banks 2i%8, 2i%8+1.
        # chunk c=2q+t of a slice -> partitions t*64:(t+1)*64 of bank q, so
        # bank q holds pixel chunks 2q and 2q+1 stacked in the partition dim
        # and one 128-partition ACT covers 1024 pixels per 512 columns.
        ps = stack.enter_context(nc.psum_tensor([2 * OC, 8, 512], F32))
        # Per-slice input semaphores: concurrent DMAs complete out of order,
        # so one counting sem can't tell which transfer landed. Slice 0 is
        # split into two column halves (one per HWDGE ring) with separate
        # sems so its first 2 matmuls / first ACT start as early as possible.
        sx0a = stack.enter_context(nc.semaphore("s_x0a"))
        sx0b = stack.enter_context(nc.semaphore("s_x0b"))
        sx = [stack.enter_context(nc.semaphore(f"s_x{i}")) for i in range(1, NH)]
        s_w = stack.enter_context(nc.semaphore("s_w"))
        s_mm = stack.enter_context(nc.semaphore("s_mm"))
        s_act = stack.enter_context(nc.semaphore("s_act"))
        s_y = stack.enter_context(nc.semaphore("s_y"))
        block = stack.enter_context(nc.Block())

        # ACT index bookkeeping: acts 1,2 cover slice 0's halves; act i+2
        # covers slice i (i>=1). 9 acts total.
        def acts_through(i):  # s_act value once slice i is fully activated
            return 2 if i == 0 else i + 2

        @block.sync
        def _(sync):
            # ring A (Sync HWDGE): slice-0 low half heads the critical path,
            # then slices 2 and 4; outputs for slices 0..6 ride behind them.
            sync.dma_start(
                out=x_bufs[:, 0, :1024], in_=xp[0][:, 0, :1024]
            ).then_inc(sx0a, 16)
            sync.dma_start(out=x_bufs[:, 2, :], in_=xp[1][:, 0, :]).then_inc(
                sx[1], 16
            )
            sync.dma_start(out=x_bufs[:, 4, :], in_=xp[2][:, 0, :]).then_inc(
                sx[3], 16
            )
            sync.dma_start(out=x_bufs[:, 6, :], in_=xp[3][:, 0, :]).then_inc(
                sx[5], 16
            )
            for i in range(7):
                sync.wait_ge(s_act, acts_through(i))
                sync.dma_start(out=y[i], in_=a_bufs[:, i]).then_inc(s_y, 16)
            sync.wait_ge(s_y, 16 * NH)

        @block.gpsimd
        def _(gpsimd):
            # ring C (SWDGE): late slices — the ~2us SWDGE latency is hidden
            # because these aren't consumed until ~5-10us in.
            gpsimd.dma_start(out=x_bufs[:, 3, :], in_=xp[1][:, 1, :]).then_inc(
                sx[2], 16
            )
            gpsimd.dma_start(out=x_bufs[:, 5, :], in_=xp[2][:, 1, :]).then_inc(
                sx[4], 16
            )
            gpsimd.dma_start(out=x_bufs[:, 7, :], in_=xp[3][:, 1, :]).then_inc(
                sx[6], 16
            )

        @block.tensor
        def _(tensor):
            # slice 0, first half (chunks 0,1 -> bank 0)
            tensor.wait_ge(s_w, 16)
            tensor.wait_ge(sx0a, 16)
            nc.tensor.matmul(
                ps[:OC, 0, :], w_tile[:], x_bufs[:, 0, 0:512], start=True, stop=True
            )
            nc.tensor.matmul(
                ps[OC:, 0, :], w_tile[:], x_bufs[:, 0, 512:1024], start=True, stop=True
            ).then_inc(s_mm, 1)
            # slice 0, second half (chunks 2,3 -> bank 1)
            tensor.wait_ge(sx0b, 16)
            nc.tensor.matmul(
                ps[:OC, 1, :], w_tile[:], x_bufs[:, 0, 1024:1536], start=True, stop=True
            )
            nc.tensor.matmul(
                ps[OC:, 1, :], w_tile[:], x_bufs[:, 0, 1536:2048], start=True, stop=True
            ).then_inc(s_mm, 1)
            for i in range(1, NH):
                if i >= 4:
                    # psum bank pair reused; wait until the ACT(s) of slice
                    # i-4 have read it
                    tensor.wait_ge(s_act, acts_through(i - 4))
                tensor.wait_ge(sx[i - 1], 16)
                last = None
                for t in range(2):
                    for q in range(2):
                        c = 2 * q + t
                        last = nc.tensor.matmul(
                            ps[t * OC : (t + 1) * OC, (2 * i + q) % 8, :],
                            w_tile[:],
                            x_bufs[:, i, c * 512 : (c + 1) * 512],
                            start=True,
                            stop=True,
                        )
                last.then_inc(s_mm, 1)

        @block.scalar
        def _(scalar):
            # ring B (Scalar HWDGE): weights first (tiny, needed by the first
            # matmul), then slice-0 high half and slice 1 — light enough that
            # the ACT table load + first ACT aren't delayed past the data.
            scalar.dma_start(out=w_tile[:], in_=w[:]).then_inc(s_w, 16)
            scalar.dma_start(
                out=x_bufs[:, 0, 1024:2048], in_=xp[0][:, 0, 1024:2048]
            ).then_inc(sx0b, 16)
            scalar.dma_start(out=x_bufs[:, 1, :], in_=xp[0][:, 1, :]).then_inc(
                sx[0], 16
            )
            # slice 0 tanh in two 512-col halves for an early chain start
            scalar.wait_ge(s_mm, 1)
            nc.scalar.activation(
                a_bufs[:, 0, :512],
                ps[:, 0, :],
                mybir.ActivationFunctionType.Tanh,
            ).then_inc(s_act, 1)
            scalar.wait_ge(s_mm, 2)
            nc.scalar.activation(
                a_bufs[:, 0, 512:],
                ps[:, 1, :],
                mybir.ActivationFunctionType.Tanh,
            ).then_inc(s_act, 1)
            for i in range(1, NH):
                scalar.wait_ge(s_mm, i + 2)
                bk = (2 * i) % 8
                nc.scalar.activation(
                    a_bufs[:, i],
                    ps[:, bk : bk + 2, :].rearrange("p b c -> p (b c)"),
                    mybir.ActivationFunctionType.Tanh,
                ).then_inc(s_act, 1)
            # last slice's store goes out on ring B the moment its ACT is
            # done (ring A may still be draining slice 6's store)
            scalar.wait_ge(s_act, acts_through(NH - 1))
            nc.scalar.dma_start(out=y[NH - 1], in_=a_bufs[:, NH - 1]).then_inc(
                s_y, 16
            )

    nc.finalize()
    return nc


def _get_program():
    global _PROGRAM
    if _PROGRAM is None:
        _PROGRAM = build_program()
    return _PROGRAM


def _im2col(x: np.ndarray) -> np.ndarray:
    """[B,8,256,256] fp32 -> [B,80,4096] fp16 patches, p=(ky*3+kx)*8+ic,
    rows 72..79 zero (padding for 16-SDMA-engine DMA spread)."""
    B, C, H, W = x.shape
    xh = x.astype(np.float16)
    xpad = np.zeros((B, C, H + 2, W + 2), np.float16)
    xpad[:, :, 1 : H + 1, 1 : W + 1] = xh
    s = xpad.strides
    # windows[b,c,ky,kx,y,x] = xpad[b,c,4y+ky,4x+kx] = x[b,c,4y+ky-1,4x+kx-1]
    win = np.lib.stride_tricks.as_strided(
        xpad,
        shape=(B, C, KH, KW, OH, OW),
        strides=(s[0], s[1], s[2], s[3], 4 * s[2], 4 * s[3]),
    )
    out = np.zeros((B, KP, NPIX), np.float16)
    np.copyto(
        out[:, :K].reshape(B, KH, KW, C, OH, OW), win.transpose(0, 2, 3, 1, 4, 5)
    )
    out[:, K] = np.float16(1.0)  # bias row: w row K carries the bias
    return out


def run_sharded(x, weight, bias, **spmd_kwargs):
    """Returns (output, BassKernelResults). spmd_kwargs e.g. trace=True."""
    patches = _im2col(x)  # [32, 80, 4096] f16, contiguous
    w_mat = np.zeros((KP, OC), np.float16)
    w_mat[:K] = weight.transpose(2, 3, 1, 0).reshape(K, OC).astype(np.float16)
    w_mat[K] = bias.astype(np.float16).reshape(OC)

    in_maps = [
        {
            "xp": patches[c * B_CORE : (c + 1) * B_CORE].reshape(B_CORE, KP, 2, HALF),
            "w": w_mat,
        }
        for c in range(N_CORES)
    ]
    nc = _get_program()
    res = run_bass_kernel_spmd(nc, in_maps, list(range(N_CORES)), **spmd_kwargs)
    # y core shard: [8 slices, 128, 1024]; slice i = (image i//2, half
    # i%2); partition p = t*64+oc; column = q*512+col; pixel chunk = 4h+2q+t
    y16 = np.concatenate([r["y"] for r in res.results], axis=0)  # [64,128,1024]
    y16 = (
        y16.reshape(B_FULL, 2, 2, OC, 2, 512)  # [b, h, t, oc, q, col]
        .transpose(0, 3, 1, 4, 2, 5)  # [b, oc, h, q, t, col]
        .reshape(B_FULL, OC, NPIX)
    )
    # 2*tanh in fp16 then cast to fp32 == cast then *2 (exact: *2 is an
    # exponent bump, in-range for |tanh|<=1)
    out = y16.astype(np.float32).reshape(B_FULL, OC, OH, OW) * np.float32(2.0)
    return out, res


def kernel(x: np.ndarray, weight: np.ndarray, bias: np.ndarray) -> np.ndarray:
    return run_sharded(x, weight, bias)[0]


# revision 9
# speedup vs baseline: 1.1771x; 1.1771x over previous
# Fused conv3x3(same) + bias + tanh + x2 + stride-4 subsample, data-parallel
# over 8 NeuronCores.
#
# Math: out[b,oc,y,x] = 2*tanh(sum_{ic,ky,kx} w[oc,ic,ky,kx]*x[b,ic,4y+ky-1,4x+kx-1] + bias[oc])
# computed in fp16 like the reference. Since the spatial stride (4) exceeds the
# kernel size (3), every output pixel reads a disjoint 3x3x8 input patch, so the
# conv lowers exactly to a [72 -> 64] GEMM over 64*64 pixels per image. The host
# does the im2col rearrangement (pure data movement, fp16 cast is identical to
# the reference's .astype(float16)); each core runs the GEMM + bias + tanh for
# 4 of the 32 images. The trailing *2 and fp32 cast are exact in either order,
# so they are applied on the host after the fp16 tanh.
#
# Device kernel is hand-scheduled raw bacc. Schedule notes (from perfetto
# trace analysis):
#  - HBM->SBUF reads cap at ~245 GB/s aggregate regardless of how many DGE
#    rings are used (measured: 1 ring = 245, 3 rings = 245 with bad
#    starvation), so ALL input slices go on the Sync HWDGE ring in
#    consumption order - ring FIFO order doubles as priority. HBM writes
#    ride on top of the read cap (mixed phases measured ~405 GB/s), so
#    outputs go out on the GpSimd SWDGE ring while inputs stream.
#  - The scalar ACT chain (tanh at 1 col/cycle, (N+352)/1.2GHz per call) is
#    ~9.8us and roughly matches the 10.7us input stream; both head and tail
#    slices are split in half so the chain starts as early as possible and
#    the last output leaves as early as possible.
#  - No PE warm-up matmuls: HAM grants full clock (2.4GHz) only after ~4us
#    of sustained PE activity and budgets ~7us of it; burning the budget on
#    warm-ups re-throttles the real matmuls to 50% duty. Real matmuls at the
#    cold 1.2GHz clock keep up with the 245 GB/s input stream after the
#    first couple of slices.
#  - The bias rides in w row K (patch row K is constant 1.0), so there is
#    no separate bias operand anywhere.
#  - The contraction is zero-padded 72 -> 80 rows: an 80-partition DMA
#    spreads evenly over all 16 SDMA engines.
import sys

import numpy as np

try:
    import concourse.bass as bass  # noqa: F401
except ImportError:
    sys.path.insert(0, "/opt/trn_rl_repo")

import concourse.bass as bass  # noqa: F401
import concourse.bacc as bacc
import concourse.mybir as mybir
from concourse.bass_utils import run_bass_kernel_spmd

N_CORES = 8
B_FULL = 32
B_CORE = B_FULL // N_CORES  # 4 images per core
C_IN = 8
KH = KW = 3
K = C_IN * KH * KW  # 72 contraction
KP = 80  # zero-padded contraction (16-SDMA-engine alignment)
OC = 64
OH = OW = 64
NPIX = OH * OW  # 4096
HALF = NPIX // 2  # 2048
NH = 2 * B_CORE  # 8 half-image pipeline slices
F16 = mybir.dt.float16
F32 = mybir.dt.float32

_PROGRAM = None


def build_program():
    from contextlib import ExitStack

    nc = bacc.Bacc("TRN2")
    xp = nc.dram_tensor("xp", [B_CORE, KP, 2, HALF], F16, kind="ExternalInput")
    w = nc.dram_tensor("w", [KP, OC], F16, kind="ExternalInput")
    y = nc.dram_tensor("y", [NH, 2 * OC, HALF // 2], F16, kind="ExternalOutput")

    with ExitStack() as stack:
        w_tile = stack.enter_context(nc.sbuf_tensor([KP, OC], F16))
        # one buffer per slice -> no buffer-reuse waits
        x_bufs = stack.enter_context(nc.sbuf_tensor([KP, NH, HALF], F16))
        a_bufs = stack.enter_context(nc.sbuf_tensor([2 * OC, NH, HALF // 2], F16))
        # 8 banks of [128, 512]; slice i accumulates into banks 2i%8, 2i%8+1.
        # chunk c=2q+t of a slice -> partitions t*64:(t+1)*64 of bank q, so
        # bank q holds pixel chunks 2q and 2q+1 stacked in the partition dim
        # and one 128-partition ACT covers 1024 pixels per 512 columns.
        ps = stack.enter_context(nc.psum_tensor([2 * OC, 8, 512], F32))
        # Per-piece input semaphores: concurrent DMAs complete out of order,
        # so one counting sem can't tell which transfer landed. Slices 0 and
        # 7 are split into two column halves (head: early first ACT; tail:
        # early last store).
        sxa = [stack.enter_context(nc.semaphore(f"s_xa{i}")) for i in range(NH)]
        sxb0 = stack.enter_context(nc.semaphore("s_xb0"))
        sxb7 = stack.enter_context(nc.semaphore("s_xb7"))
        s_w = stack.enter_context(nc.semaphore("s_w"))
        s_mm = stack.enter_context(nc.semaphore("s_mm"))
        s_act = stack.enter_context(nc.semaphore("s_act"))
        s_y = stack.enter_context(nc.semaphore("s_y"))
        s_y2 = stack.enter_context(nc.semaphore("s_y2"))
        block = stack.enter_context(nc.Block())

        # ACT chain: acts 1,2 = slice 0's bank halves; act i+2 = slice i
        # (1<=i<=6); acts 9,10 = slice 7's bank halves. 10 acts total.
        def acts_through(i):  # s_act value once slice i is fully activated
            return 2 if i == 0 else (10 if i == 7 else i + 2)

        @block.sync
        def _(sync):
            # all inputs on one ring, in consumption order (FIFO = priority)
            sync.dma_start(
                out=x_bufs[:, 0, :1024], in_=xp[0][:, 0, :1024]
            ).then_inc(sxa[0], 16)
            sync.dma_start(
                out=x_bufs[:, 0, 1024:], in_=xp[0][:, 0, 1024:]
            ).then_inc(sxb0, 16)
            for i in range(1, NH - 1):
                sync.dma_start(
                    out=x_bufs[:, i, :], in_=xp[i // 2][:, i % 2, :]
                ).then_inc(sxa[i], 16)
            sync.dma_start(
                out=x_bufs[:, 7, :1024], in_=xp[3][:, 1, :1024]
            ).then_inc(sxa[7], 16)
            sync.dma_start(
                out=x_bufs[:, 7, 1024:], in_=xp[3][:, 1, 1024:]
            ).then_inc(sxb7, 16)
            sync.wait_ge(s_y, 16 * 8)
            sync.wait_ge(s_y2, 16)

        @block.gpsimd
        def _(gpsimd):
            # stores ride the SWDGE ring so they never contend with the
            # input ring's descriptor stream; writes overlap the read cap.
            for i in range(NH - 1):
                gpsimd.wait_ge(s_act, acts_through(i))
                gpsimd.dma_start(out=y[i], in_=a_bufs[:, i]).then_inc(s_y, 16)
            gpsimd.wait_ge(s_act, 9)
            gpsimd.dma_start(out=y[7][:, :512], in_=a_bufs[:, 7, :512]).then_inc(
                s_y, 16
            )

        @block.tensor
        def _(tensor):
            # slice 0, first half (chunks 0,1 -> bank 0)
            tensor.wait_ge(s_w, 16)
            tensor.wait_ge(sxa[0], 16)
            nc.tensor.matmul(
                ps[:OC, 0, :], w_tile[:], x_bufs[:, 0, 0:512], start=True, stop=True
            )
            nc.tensor.matmul(
                ps[OC:, 0, :], w_tile[:], x_bufs[:, 0, 512:1024], start=True, stop=True
            ).then_inc(s_mm, 1)
            # slice 0, second half (chunks 2,3 -> bank 1)
            tensor.wait_ge(sxb0, 16)
            nc.tensor.matmul(
                ps[:OC, 1, :], w_tile[:], x_bufs[:, 0, 1024:1536], start=True, stop=True
            )
            nc.tensor.matmul(
                ps[OC:, 1, :], w_tile[:], x_bufs[:, 0, 1536:2048], start=True, stop=True
            ).then_inc(s_mm, 1)
            for i in range(1, NH - 1):
                if i >= 4:
                    # psum bank pair reused; wait until the ACT(s) of slice
                    # i-4 have read it
                    tensor.wait_ge(s_act, acts_through(i - 4))
                tensor.wait_ge(sxa[i], 16)
                last = None
                for t in range(2):
                    for q in range(2):
                        c = 2 * q + t
                        last = nc.tensor.matmul(
                            ps[t * OC : (t + 1) * OC, (2 * i + q) % 8, :],
                            w_tile[:],
                            x_bufs[:, i, c * 512 : (c + 1) * 512],
                            start=True,
                            stop=True,
                        )
                last.then_inc(s_mm, 1)
            # slice 7 (banks 6,7; bank pair last used by slice 3)
            tensor.wait_ge(s_act, acts_through(3))
            tensor.wait_ge(sxa[7], 16)
            nc.tensor.matmul(
                ps[:OC, 6, :], w_tile[:], x_bufs[:, 7, 0:512], start=True, stop=True
            )
            nc.tensor.matmul(
                ps[OC:, 6, :], w_tile[:], x_bufs[:, 7, 512:1024], start=True, stop=True
            ).then_inc(s_mm, 1)
            tensor.wait_ge(sxb7, 16)
            nc.tensor.matmul(
                ps[:OC, 7, :], w_tile[:], x_bufs[:, 7, 1024:1536], start=True, stop=True
            )
            nc.tensor.matmul(
                ps[OC:, 7, :], w_tile[:], x_bufs[:, 7, 1536:2048], start=True, stop=True
            ).then_inc(s_mm, 1)

        @block.scalar
        def _(scalar):
            # w is tiny and the scalar HWDGE ring is otherwise idle; it lands
            # well before the first matmul needs it.
            scalar.dma_start(out=w_tile[:], in_=w[:]).then_inc(s_w, 16)
            # tanh chain: 10 ACTs (slices 0 and 7 in bank halves)
            scalar.wait_ge(s_mm, 1)
            nc.scalar.activation(
                a_bufs[:, 0, :512], ps[:, 0, :], mybir.ActivationFunctionType.Tanh
            ).then_inc(s_act, 1)
            scalar.wait_ge(s_mm, 2)
            nc.scalar.activation(
                a_bufs[:, 0, 512:], ps[:, 1, :], mybir.ActivationFunctionType.Tanh
            ).then_inc(s_act, 1)
            for i in range(1, NH - 1):
                scalar.wait_ge(s_mm, i + 2)
                bk = (2 * i) % 8
                nc.scalar.activation(
                    a_bufs[:, i],
                    ps[:, bk : bk + 2, :].rearrange("p b c -> p (b c)"),
                    mybir.ActivationFunctionType.Tanh,
                ).then_inc(s_act, 1)
            scalar.wait_ge(s_mm, 9)
            nc.scalar.activation(
                a_bufs[:, 7, :512], ps[:, 6, :], mybir.ActivationFunctionType.Tanh
            ).then_inc(s_act, 1)
            scalar.wait_ge(s_mm, 10)
            nc.scalar.activation(
                a_bufs[:, 7, 512:], ps[:, 7, :], mybir.ActivationFunctionType.Tanh
            ).then_inc(s_act, 1)
            # last store leaves on the idle scalar HWDGE ring immediately
            scalar.wait_ge(s_act, 10)
            nc.scalar.dma_start(
                out=y[7][:, 512:], in_=a_bufs[:, 7, 512:]
            ).then_inc(s_y2, 16)

    nc.finalize()
    return nc


def _get_program():
    global _PROGRAM
    if _PROGRAM is None:
        _PROGRAM = build_program()
    return _PROGRAM


def _im2col(x: np.ndarray) -> np.ndarray:
    """[B,8,256,256] fp32 -> [B,80,4096] fp16 patches, p=(ky*3+kx)*8+ic,
    rows 72..79 zero (padding for 16-SDMA-engine DMA spread)."""
    B, C, H, W = x.shape
    xh = x.astype(np.float16)
    xpad = np.zeros((B, C, H + 2, W + 2), np.float16)
    xpad[:, :, 1 : H + 1, 1 : W + 1] = xh
    s = xpad.strides
    # windows[b,c,ky,kx,y,x] = xpad[b,c,4y+ky,4x+kx] = x[b,c,4y+ky-1,4x+kx-1]
    win = np.lib.stride_tricks.as_strided(
        xpad,
        shape=(B, C, KH, KW, OH, OW),
        strides=(s[0], s[1], s[2], s[3], 4 * s[2], 4 * s[3]),
    )
    out = np.zeros((B, KP, NPIX), np.float16)
    np.copyto(
        out[:, :K].reshape(B, KH, KW, C, OH, OW), win.transpose(0, 2, 3, 1, 4, 5)
    )
    out[:, K] = np.float16(1.0)  # bias row: w row K carries the bias
    return out


def run_sharded(x, weight, bias, **spmd_kwargs):
    """Returns (output, BassKernelResults). spmd_kwargs e.g. trace=True."""
    patches = _im2col(x)  # [32, 80, 4096] f16, contiguous
    w_mat = np.zeros((KP, OC), np.float16)
    w_mat[:K] = weight.transpose(2, 3, 1, 0).reshape(K, OC).astype(np.float16)
    w_mat[K] = bias.astype(np.float16).reshape(OC)

    in_maps = [
        {
            "xp": patches[c * B_CORE : (c + 1) * B_CORE].reshape(B_CORE, KP, 2, HALF),
            "w": w_mat,
        }
        for c in range(N_CORES)
    ]
    nc = _get_program()
    res = run_bass_kernel_spmd(nc, in_maps, list(range(N_CORES)), **spmd_kwargs)
    # y core shard: [8 slices, 128, 1024]; slice i = (image i//2, half
    # i%2); partition p = t*64+oc; column = q*512+col; pixel chunk = 4h+2q+t
    y16 = np.concatenate([r["y"] for r in res.results], axis=0)  # [64,128,1024]
    y16 = (
        y16.reshape(B_FULL, 2, 2, OC, 2, 512)  # [b, h, t, oc, q, col]
        .transpose(0, 3, 1, 4, 2, 5)  # [b, oc, h, q, t, col]
        .reshape(B_FULL, OC, NPIX)
    )
    # 2*tanh in fp16 then cast to fp32 == cast then *2 (exact: *2 is an
    # exponent bump, in-range for |tanh|<=1)
    out = y16.astype(np.float32).reshape(B_FULL, OC, OH, OW) * np.float32(2.0)
    return out, res


def kernel(x: np.ndarray, weight: np.ndarray, bias: np.ndarray) -> np.ndarray:
    return run_sharded(x, weight, bias)[0]
